# revision 7
# baseline (speedup 1.0000x reference)
"""Trainium2 Bass kernel for the CrossLayer problem.

Math: reference computes, per row x (length D), with cur_0 = x:
    cur_{i+1} = sum(cur_i) * (w_i ⊙ x) + b_i + x        (i = 0..L-1)
Only the scalar s_i = sum(cur_i) couples elements, so with
    X   = sum(x)                  (per row)
    W_i = x · w_i                 (per row, i = 0..L-2)
    c_i = sum(b_i)
the recursion collapses to scalars:
    S_0 = X;  S_{i+1} = S_i * W_i + c_i + X
and the output is a single elementwise pass:
    out = S_{L-1} * (w_{L-1} ⊙ x) + b_{L-1} + x
        = x ⊙ (S_{L-1} * w_{L-1} + 1) + b_{L-1}

Kernel layout (per core, pure data parallel over batch):
  - rows on partitions, 16 tiles of (128, 1024) f32, processed in PAIRS
  - PE transposes each 128x128 chunk of both tiles into 4 PSUM banks per
    pair; ACT/DVE copy PSUM->SBUF (split 2/2 to balance engine load)
  - dots [X, W0, W1, W2] via 8 accumulating matmuls with N=256 moving
    (both tiles of the pair) in float32r: 1 cycle/row instead of fp32's 4
  - small PE transposes put the dots row-major; DVE runs the scalar
    recursion (tensor_scalar with per-partition scale/addend)
  - t = S3*w3 + 1 on ACT (activation Copy with per-partition scale,
    immediate bias); final out = t ⊙ x is a single DVE pass per tile
  - x-in DMAs ride the scalar (ACT) HWDGE queue, out DMAs the sync (SP)
    queue, so output stores never head-of-line-block input loads
"""

import os
import numpy as np

B, D, L = 16384, 1024, 4
N_CORES = 8
RPC = B // N_CORES          # rows per core
P = 128                     # partitions
N_TILES = RPC // P          # 16
N_PAIRS = N_TILES // 2      # 8
N_CHUNKS = D // P           # 8

_built = {}


def _build_nc(b_zero: bool):
    import concourse.bass as bass
    import concourse.bacc as bacc
    import concourse.mybir as mybir
    from concourse import tile

    f32 = mybir.dt.float32
    f32r = mybir.dt.float32r
    Alu = mybir.AluOpType
    Act = mybir.ActivationFunctionType

    # Bacc (not raw Bass): its compile() legalizes semaphore waits — TRN2
    # matmuls encode at most one sync wait (walrus S3_LW struct).
    nc = bacc.Bacc(
        "TRN2", target_bir_lowering=False, debug=False, num_devices=N_CORES
    )
    x_d = nc.dram_tensor("x", [RPC, D], f32, kind="ExternalInput")
    wpk_d = nc.dram_tensor("wpk", [P, N_CHUNKS * 4], f32r, kind="ExternalInput")
    w3bc_d = nc.dram_tensor("w3bc", [P, D], f32, kind="ExternalInput")
    ident_d = nc.dram_tensor("ident", [P, P], f32, kind="ExternalInput")
    if not b_zero:
        cvec_d = nc.dram_tensor("cvec", [P, 4], f32, kind="ExternalInput")
        b3bc_d = nc.dram_tensor("b3bc", [P, D], f32, kind="ExternalInput")
    out_d = nc.dram_tensor("out", [RPC, D], f32, kind="ExternalOutput")

    with tile.TileContext(nc) as tc:
        with (
            tc.tile_pool(name="consts", bufs=1) as consts,
            tc.tile_pool(name="xin", bufs=10) as xin_pool,
            tc.tile_pool(name="xts", bufs=2) as xts_pool,
            tc.tile_pool(name="tp", bufs=4) as t_pool,
            tc.tile_pool(name="outp", bufs=6) as out_pool,
            tc.tile_pool(name="small", bufs=3) as small_pool,
            tc.tile_pool(name="ps_t", bufs=4, space=bass.MemorySpace.PSUM) as ps_t,
            tc.tile_pool(name="ps_d", bufs=2, space=bass.MemorySpace.PSUM) as ps_d,
            tc.tile_pool(name="ps_s", bufs=2, space=bass.MemorySpace.PSUM) as ps_s,
        ):
            # x-ins on the scalar (ACT) HWDGE queue; outs on sync. Keep the
            # in-stream PRE tiles ahead so PE never starves. First tiles'
            # loads split in halves so the transpose pipeline fills sooner.
            PRE = 6
            pre_xts = {}

            def load_xt(t, split):
                eng = nc.scalar
                xt = xin_pool.tile([P, D], f32, name="xt")
                if split:
                    eng.dma_start(
                        xt[:, 0:D // 2], x_d[t * P:(t + 1) * P, 0:D // 2]
                    )
                    eng.dma_start(
                        xt[:, D // 2:D], x_d[t * P:(t + 1) * P, D // 2:D]
                    )
                else:
                    eng.dma_start(xt[:], x_d[t * P:(t + 1) * P, :])
                pre_xts[t] = xt

            for t in range(3):
                load_xt(t, split=True)
            for t in range(3, PRE):
                load_xt(t, split=False)

            # consts on the sync queue (idle until the first out-DMA)
            wpk = consts.tile([P, N_CHUNKS * 4], f32r)
            nc.sync.dma_start(wpk[:], wpk_d[:])
            w3bc = consts.tile([P, D], f32)
            nc.sync.dma_start(w3bc[:], w3bc_d[:])
            ident = consts.tile([P, P], f32)
            nc.sync.dma_start(ident[:], ident_d[:])
            if not b_zero:
                cvec = consts.tile([P, 4], f32)
                nc.sync.dma_start(cvec[:], cvec_d[:])
                b3bc = consts.tile([P, D], f32)
                nc.sync.dma_start(b3bc[:], b3bc_d[:])

            # Prologue: absorb each const-DMA completion into one engine
            # observation up front, so steady-state instructions never need
            # two fresh semaphore waits (walrus: one sync wait per matmul).
            prol0 = ps_t.tile([P, D // 2], f32, name="prol0", tag="xt_ps")
            nc.tensor.transpose(prol0[0:P, 0:P], ident[:], ident[:])
            prol1 = ps_d.tile([4, 2 * P], f32, name="prol1", tag="dots_ps")
            nc.tensor.matmul(
                prol1[:, 0:32],
                wpk[:, 0:4],
                wpk[:],
                start=True,
                stop=True,
            )
            prolc = small_pool.tile([P, 1], f32, name="prolc")
            nc.scalar.activation(prolc[:], w3bc[:, 0:1], Act.Copy)
            prolv = small_pool.tile([P, 1], f32, name="prolv")
            nc.vector.tensor_mul(prolv[:], w3bc[:, 0:1], w3bc[:, 0:1])
            if not b_zero:
                prolc2 = small_pool.tile([P, 1], f32, name="prolc2")
                nc.vector.tensor_mul(prolc2[:], cvec[:, 0:1], cvec[:, 0:1])
                prolb = small_pool.tile([P, 1], f32, name="prolb")
                nc.vector.tensor_mul(prolb[:], b3bc[:, 0:1], b3bc[:, 0:1])

            for p in range(N_PAIRS):
                t0, t1 = 2 * p, 2 * p + 1
                x0, x1 = pre_xts[t0], pre_xts[t1]
                for nxt in (t0 + PRE, t1 + PRE):
                    if nxt < N_TILES:
                        load_xt(nxt, split=False)

                # xts[:, c*256:(c+1)*256] = [xT of t0 chunk c | xT of t1
                # chunk c]; built via 4 PSUM banks of 2 chunks each
                xts = xts_pool.tile([P, 4 * 512], f32r, name="xts")
                for k in range(4):
                    xt_ps = ps_t.tile([P, 512], f32, name="xt_ps", tag="xt_ps")
                    for h, xt in ((0, x0), (1, x1)):
                        for cc in range(2):
                            c = 2 * k + cc
                            nc.tensor.transpose(
                                xt_ps[:, cc * 256 + h * P:cc * 256 + (h + 1) * P],
                                xt[:, c * P:(c + 1) * P],
                                ident[:],
                            )
                    dst = xts[:, k * 512:(k + 1) * 512]
                    if k < 2:
                        nc.scalar.copy(dst, xt_ps[:])
                    else:
                        nc.vector.tensor_copy(dst, xt_ps[:])

                # dots[i, n] = [X, W0, W1, W2] for rows n<128: t0, n>=128: t1
                # f32r with N=256 streams 1 cycle/row (fp32 is 4)
                dots_ps = ps_d.tile([4, 2 * P], f32, name="dots_ps", tag="dots_ps")
                for c in range(N_CHUNKS):
                    nc.tensor.matmul(
                        dots_ps[:],
                        wpk[:, c * 4:(c + 1) * 4],
                        xts[:, c * 256:(c + 1) * 256],
                        start=(c == 0),
                        stop=(c == N_CHUNKS - 1),
                    )
                dots = small_pool.tile([4, 2 * P], f32, name="dots")
                nc.vector.tensor_copy(dots[:], dots_ps[:])

                # back to row-major: dT[r, 4*h + i] for tile h
                dT_ps = ps_s.tile([P, 8], f32, name="dT_ps")
                for h in range(2):
                    nc.tensor.transpose(
                        dT_ps[:, h * 4:(h + 1) * 4],
                        dots[:, h * P:(h + 1) * P],
                        ident[0:4, 0:4],
                    )
                dT = small_pool.tile([P, 8], f32, name="dT")
                nc.vector.tensor_copy(dT[:], dT_ps[:])

                # scalar recursion S_{i+1} = S_i * W_i + (X + c_i) per tile
                svec = small_pool.tile([P, 8], f32, name="svec")
                if not b_zero:
                    avec = small_pool.tile([P, 8], f32, name="avec")
                for h, xt in ((0, x0), (1, x1)):
                    X = dT[:, 4 * h:4 * h + 1]
                    if b_zero:
                        addends = [X, X, X]
                    else:
                        for i in range(3):
                            nc.vector.tensor_scalar_add(
                                avec[:, 4 * h + i:4 * h + i + 1],
                                X,
                                cvec[:, i:i + 1],
                            )
                        addends = [
                            avec[:, 4 * h + i:4 * h + i + 1] for i in range(3)
                        ]
                    s_prev = X
                    for i in range(3):
                        nc.vector.tensor_scalar(
                            svec[:, 4 * h + i:4 * h + i + 1],
                            s_prev,
                            dT[:, 4 * h + i + 1:4 * h + i + 2],
                            addends[i],
                            Alu.mult,
                            Alu.add,
                        )
                        s_prev = svec[:, 4 * h + i:4 * h + i + 1]
                    S3 = svec[:, 4 * h + 2:4 * h + 3]

                    # t = S3 * w3 + 1 on ACT; out = t ⊙ x on DVE
                    t_sb = t_pool.tile([P, D], f32, name="t_sb")
                    nc.scalar.activation(
                        t_sb[:], w3bc[:], Act.Copy, bias=1.0, scale=S3
                    )
                    out_sb = out_pool.tile([P, D], f32, name="out_sb")
                    nc.vector.tensor_mul(out_sb[:], t_sb[:], xt[:])
                    if not b_zero:
                        out2 = out_pool.tile([P, D], f32, name="out2")
                        nc.vector.tensor_add(out2[:], out_sb[:], b3bc[:])
                        out_sb = out2
                    t = 2 * p + h
                    nc.sync.dma_start(out_d[t * P:(t + 1) * P, :], out_sb[:])
    nc.compile()
    return nc


def _get_nc(b_zero: bool):
    if b_zero not in _built:
        _built[b_zero] = _build_nc(b_zero)
    return _built[b_zero]


def _host_prep(w, b, b_zero):
    # Wpk[p, c*4+i] packs column i of [ones, w0, w1, w2] for D-chunk c
    M = np.empty((D, 4), dtype=np.float32)
    M[:, 0] = 1.0
    M[:, 1] = w[0]
    M[:, 2] = w[1]
    M[:, 3] = w[2]
    wpk = np.ascontiguousarray(
        M.reshape(N_CHUNKS, P, 4).transpose(1, 0, 2).reshape(P, N_CHUNKS * 4)
    )
    w3bc = np.ascontiguousarray(np.broadcast_to(w[3], (P, D)).astype(np.float32))
    ident = np.eye(P, dtype=np.float32)
    extras = {}
    if not b_zero:
        c = b.sum(axis=1).astype(np.float32)  # (L,)
        extras["cvec"] = np.ascontiguousarray(np.broadcast_to(c, (P, L)))
        extras["b3bc"] = np.ascontiguousarray(
            np.broadcast_to(b[3], (P, D)).astype(np.float32)
        )
    return wpk, w3bc, ident, extras


def kernel(inputs, w, b):
    from concourse.bass_utils import run_bass_kernel_spmd

    x = np.ascontiguousarray(np.asarray(inputs, dtype=np.float32).reshape(B, D))
    w = np.asarray(w, dtype=np.float32)
    b = np.asarray(b, dtype=np.float32)
    b_zero = not b.any()

    nc = _get_nc(b_zero)
    wpk, w3bc, ident, extras = _host_prep(w, b, b_zero)

    in_maps = []
    for i in range(N_CORES):
        m = {
            "x": x[i * RPC:(i + 1) * RPC],
            "wpk": wpk,
            "w3bc": w3bc,
            "ident": ident,
        }
        m.update(extras)
        in_maps.append(m)

    trace = bool(int(os.environ.get("KERNEL_TRACE", "0")))
    kwargs = {}
    if trace:
        kwargs = {"trace": True, "trace_cores": [0]}
    res = run_bass_kernel_spmd(nc, in_maps, core_ids=list(range(N_CORES)), **kwargs)
    if trace:
        kernel.last_results = res
    return np.concatenate([r["out"] for r in res.results], axis=0)
